# revision 19
# baseline (speedup 1.0000x reference)
"""Trainium2 Bass kernel for multi-head attention (nn_AttentionWithDropout).

Reference computation (fp32):
    q = query @ Wq.T + bq ; k = key @ Wk.T + bk ; v = value @ Wv.T + bv
    per head: P = softmax(q k^T / sqrt(E)) ; o = P v
    out = concat_heads(o) @ Wo.T + bo

Sharding (8 cores): data-parallel over batch (2 groups of 4 cores) x
tensor-parallel over heads (4 heads / 256 channels per core, Megatron
column-sharded Wq/Wk/Wv).  Each core computes attention output transposed
(aoT, [chans, tok]) for its heads, an AllGather within the 4-core batch
group collects the full aoT [E, L], and each core then computes the
complete output for a 256-wide slice of output channels with a full
contraction over E (fc_out sharded over output columns; bias sliced the
same way).  The host only concatenates the per-core output shards.

Softmax skips the max-subtraction: energies are ~N(0, 0.25^2) for this
problem (|energy| < ~1.5), so exp() is numerically safe, and the row sum
is produced by an extra all-ones column appended to V (so PV matmul gives
[o | r] in one accumulation); normalization multiplies by 1/r.
"""

import os
import sys

sys.path.insert(0, "/opt/trn_rl_repo")

import numpy as np

# ---- problem constants (hardcoded per the harness contract) ----
B, L, E = 2, 2048, 1024
H, D = 16, 64
N_CORES = 8
TP = 4                  # cores per batch group (head-parallel)
CH = E // TP            # 256 channels (4 heads) per core
LQ = L // TP            # 512 output tokens per core
SCALE = 1.0 / 32.0      # 1/sqrt(E)
KT = E // 128           # 8 contraction tiles for projections
NKT = L // 128          # 16 key-token tiles

# matmul operand dtype: "f32" (exact, 4 cyc/row) or "f32r" (fast fp32, 1
# cyc/row for moving dim >= 256) -- switched after precision measurement.
MM_DTYPE = os.environ.get("KERNEL_MM_DTYPE", "f32")


def _mm_ap(ap, mmdt):
    """Bitcast an fp32 AP to the matmul dtype (float32r) if requested."""
    from concourse import mybir

    if mmdt == "f32r":
        return ap.bitcast(mybir.dt.float32r)
    return ap


def _split_multi_waits(nc):
    """The nix walrus in this container only encodes one semaphore wait per
    instruction (setupSyncWait raises "Too many sync wait commands" above
    that).  Tile's wait assignment attaches several.  Hoist the extras into
    standalone InstEventSemaphore waits (the encoding `engine.wait_ge` uses)
    immediately before the owning instruction, preserving per-engine order
    and exact semantics."""
    from concourse import mybir

    n_split = 0
    for fn in nc.m.functions:
        for bb in fn.blocks:
            out = []
            for inst in bb.instructions:
                si = inst.sync_info
                if si is not None and si.on_wait and len(si.on_wait) > 1:
                    waits = list(si.on_wait)
                    for k, w in enumerate(waits[:-1]):
                        wi = mybir.InstEventSemaphore(
                            name=f"{inst.name}-hw{k}", ins=[], outs=[])
                        wi.engine = inst.engine
                        wi.debug = inst.debug
                        wi.sync_info = mybir.SyncInfo(on_wait=[w],
                                                      on_update=[])
                        out.append(wi)
                        n_split += 1
                    si.on_wait = [waits[-1]]
                out.append(inst)
            bb.instructions[:] = out
    return n_split


def _build_nc():
    import concourse.bass as bass
    import concourse.tile as tile
    from concourse import masks, mybir

    f32 = mybir.dt.float32
    AF = mybir.ActivationFunctionType

    nc = bass.Bass("TRN2", target_bir_lowering=False, debug=False,
                   num_devices=N_CORES)

    # ---- per-core external IO ----
    xqT = nc.dram_tensor("xqT", [E, L], f32, kind="ExternalInput")
    xkT = nc.dram_tensor("xkT", [E, L], f32, kind="ExternalInput")
    xvT = nc.dram_tensor("xvT", [E, L], f32, kind="ExternalInput")
    wqT = nc.dram_tensor("wqT", [E, CH], f32, kind="ExternalInput")
    wkT = nc.dram_tensor("wkT", [E, CH], f32, kind="ExternalInput")
    wvT = nc.dram_tensor("wvT", [E, CH], f32, kind="ExternalInput")
    bqc = nc.dram_tensor("bqc", [CH], f32, kind="ExternalInput")
    bkc = nc.dram_tensor("bkc", [CH], f32, kind="ExternalInput")
    bvc = nc.dram_tensor("bvc", [CH], f32, kind="ExternalInput")
    woT = nc.dram_tensor("woT", [E, CH], f32, kind="ExternalInput")
    bor = nc.dram_tensor("bor", [128, CH], f32, kind="ExternalInput")
    out = nc.dram_tensor("out", [L, CH], f32, kind="ExternalOutput")

    mmdt = MM_DTYPE

    with tile.TileContext(nc) as tc:
        with (
            tc.tile_pool(name="consts", bufs=1) as consts,
            tc.tile_pool(name="persist", bufs=1) as persist,
            tc.tile_pool(name="dram", bufs=1, space="DRAM") as dpool,
        ):
            ag_in = dpool.tile([CH, L], f32)
            ag_out = dpool.tile([TP, CH, L], f32)

            # ---- constants ----
            ones_sb = consts.tile([1, 128], f32)
            nc.vector.memset(ones_sb[:], 1.0)
            ident = consts.tile([128, 128], f32)
            masks.make_identity(nc, ident[:])
            bo_bcast = consts.tile([128, CH], f32)
            nc.sync.dma_start(bo_bcast[:], bor[:])

            bias_cols = {}
            for nm, src in (("q", bqc), ("k", bkc), ("v", bvc)):
                for ct in range(CH // 128):
                    t = consts.tile([128, 1], f32, name=f"b{nm}{ct}")
                    nc.sync.dma_start(
                        t[:], src[ct * 128:(ct + 1) * 128].unsqueeze(1))
                    bias_cols[(nm, ct)] = t

            # ---- persistent SBUF tensors ----
            qT = [persist.tile([128, L], f32, name=f"qT{i}") for i in range(2)]
            kTt = [persist.tile([128, L], f32, name=f"kT{i}") for i in range(2)]
            # v tiles: [tok 128, 4 heads x (64 v-cols + 1 ones-col)]
            v_sb = [persist.tile([128, 4, 65], f32, name=f"v{t}")
                    for t in range(NKT)]
            for t in range(NKT):
                nc.vector.memset(v_sb[t][:, :, 64:65], 1.0)
            woT_sb = [persist.tile([128, CH], f32, name=f"woT{i}")
                      for i in range(KT)]
            for i in range(KT):
                nc.sync.dma_start(woT_sb[i][:], woT[i * 128:(i + 1) * 128, :])

            # ================= projections =================
            with (
                tc.tile_pool(name="xpool", bufs=9) as xpool,
                tc.tile_pool(name="wpool", bufs=9) as wpool,
                tc.tile_pool(name="evacp", bufs=3) as evacp,
                tc.tile_pool(name="ppj", bufs=2, space="PSUM") as ppj,
                tc.tile_pool(name="pvt", bufs=2, space="PSUM") as pvt,
            ):
                for nm, xT_d, wT_d in (("q", xqT, wqT), ("k", xkT, wkT),
                                       ("v", xvT, wvT)):
                    wch = []
                    for kt in range(KT):
                        w = wpool.tile([128, CH], f32, name="wch")
                        nc.sync.dma_start(w[:], wT_d[kt * 128:(kt + 1) * 128, :])
                        wch.append(w)
                    for tc5 in range(L // 512):
                        xch = []
                        for kt in range(KT):
                            x = xpool.tile([128, 512], f32, name="xch")
                            nc.sync.dma_start(
                                x[:], xT_d[kt * 128:(kt + 1) * 128,
                                           tc5 * 512:(tc5 + 1) * 512])
                            xch.append(x)
                        for ct in range(CH // 128):
                            ps = ppj.tile([128, 512], f32, name="pj")
                            for kt in range(KT):
                                nc.tensor.matmul(
                                    ps[:],
                                    _mm_ap(wch[kt][:, ct * 128:(ct + 1) * 128], mmdt),
                                    _mm_ap(xch[kt][:], mmdt),
                                    start=(kt == 0), stop=(kt == KT - 1))
                            if nm in ("q", "k"):
                                dst = (qT if nm == "q" else kTt)[ct]
                                nc.scalar.activation(
                                    dst[:, tc5 * 512:(tc5 + 1) * 512], ps[:],
                                    AF.Identity, bias=bias_cols[(nm, ct)][:])
                            else:
                                # v: add bias, then transpose to natural
                                # [tok, chan] layout with the ones column.
                                vt = evacp.tile([128, 512], f32, name="vtmp")
                                nc.scalar.activation(
                                    vt[:], ps[:], AF.Identity,
                                    bias=bias_cols[("v", ct)][:])
                                for i in range(4):
                                    tt = tc5 * 4 + i
                                    pt = pvt.tile([128, 128], f32, name="vt")
                                    nc.tensor.transpose(
                                        pt[:], vt[:, i * 128:(i + 1) * 128],
                                        ident[:])
                                    nc.vector.tensor_copy(
                                        v_sb[tt][:, 2 * ct:2 * ct + 2, 0:64],
                                        pt.rearrange("p (h d) -> p h d", h=2))

            # ================= attention =================
            with (
                tc.tile_pool(name="upool", bufs=3) as upool,
                tc.tile_pool(name="mpool", bufs=4) as mpool,
                tc.tile_pool(name="aotp", bufs=3) as aotp,
                tc.tile_pool(name="pst", bufs=2, space="PSUM") as pst,
                tc.tile_pool(name="pacc", bufs=2, space="PSUM") as pacc,
                tc.tile_pool(name="pbc", bufs=2, space="PSUM") as pbc,
            ):
                for hp in range(2):
                    for qc in range(TP):
                        accs = [pacc.tile([65, 512], f32, name="acc")
                                for _ in range(2)]
                        for kt in range(NKT):
                            st = pst.tile([128, 1024], f32, name="st")
                            for j in range(2):
                                nc.tensor.matmul(
                                    st[:, j * 512:(j + 1) * 512],
                                    _mm_ap(kTt[hp][j * 64:(j + 1) * 64,
                                                   kt * 128:(kt + 1) * 128], mmdt),
                                    _mm_ap(qT[hp][j * 64:(j + 1) * 64,
                                                  qc * 512:(qc + 1) * 512], mmdt),
                                    start=True, stop=True)
                            u = upool.tile([128, 1024], f32, name="u")
                            nc.scalar.activation(u[:], st[:], AF.Exp,
                                                 scale=SCALE)
                            for j in range(2):
                                hl = 2 * hp + j
                                nc.tensor.matmul(
                                    accs[j][:],
                                    _mm_ap(v_sb[kt][:, hl, :], mmdt),
                                    _mm_ap(u[:, j * 512:(j + 1) * 512], mmdt),
                                    start=(kt == 0), stop=(kt == NKT - 1))
                        for j in range(2):
                            hl = 2 * hp + j
                            invr = mpool.tile([1, 512], f32, name="invr")
                            nc.vector.reciprocal(invr[:], accs[j][64:65, :])
                            bc = pbc.tile([64, 512], f32, name="bc")
                            nc.tensor.matmul(bc[:], ones_sb[:, 0:64],
                                             invr[:], start=True, stop=True)
                            bc_sb = mpool.tile([64, 512], f32, name="bcs")
                            nc.vector.tensor_copy(bc_sb[:], bc[:])
                            aoT_n = aotp.tile([64, 512], f32, name="aot")
                            nc.vector.tensor_mul(aoT_n[:], accs[j][0:64, :],
                                                 bc_sb[:])
                            nc.sync.dma_start(
                                ag_in[hl * 64:(hl + 1) * 64,
                                      qc * 512:(qc + 1) * 512],
                                aoT_n[:])

            # ================= all-gather =================
            nc.gpsimd.collective_compute(
                "AllGather", mybir.AluOpType.bypass,
                replica_groups=[[0, 1, 2, 3], [4, 5, 6, 7]],
                ins=[ag_in.opt()], outs=[ag_out.opt()])

            # ================= output projection =================
            # out[:, my 256 outchans] = aoT_full.T @ woT_slice + bo_slice
            with (
                tc.tile_pool(name="agp", bufs=9) as agp,
                tc.tile_pool(name="opool", bufs=3) as opool,
                tc.tile_pool(name="pout", bufs=2, space="PSUM") as pout,
            ):
                for tcg in range(L // 512):
                    agch = []
                    for kt in range(KT):
                        blk, row = divmod(kt, 2)
                        a = agp.tile([128, 512], f32, name="agch")
                        nc.sync.dma_start(
                            a[:], ag_out[blk, row * 128:(row + 1) * 128,
                                         tcg * 512:(tcg + 1) * 512])
                        agch.append(a)
                    for ti in range(4):
                        ps = pout.tile([128, CH], f32, name="po")
                        for kt in range(KT):
                            nc.tensor.matmul(
                                ps[:],
                                _mm_ap(agch[kt][:, ti * 128:(ti + 1) * 128], mmdt),
                                _mm_ap(woT_sb[kt][:], mmdt),
                                start=(kt == 0), stop=(kt == KT - 1))
                        ob = opool.tile([128, CH], f32, name="ob")
                        nc.vector.tensor_add(ob[:], ps[:], bo_bcast[:])
                        row0 = tcg * 512 + ti * 128
                        nc.sync.dma_start(out[row0:row0 + 128, :], ob[:])

    _split_multi_waits(nc)
    return nc


_NC_CACHE = {}


def _get_nc():
    key = MM_DTYPE
    if key not in _NC_CACHE:
        _NC_CACHE[key] = _build_nc()
    return _NC_CACHE[key]


def kernel(query, key, value, Wq, bq, Wk, bk, Wv, bv, Wo, bo,
           _trace=False, _trace_cores=None):
    from concourse.bass_utils import run_bass_kernel_spmd

    query = np.asarray(query, dtype=np.float32)
    key = np.asarray(key, dtype=np.float32)
    value = np.asarray(value, dtype=np.float32)
    Wq = np.asarray(Wq, dtype=np.float32)
    bq = np.asarray(bq, dtype=np.float32)
    Wk = np.asarray(Wk, dtype=np.float32)
    bk = np.asarray(bk, dtype=np.float32)
    Wv = np.asarray(Wv, dtype=np.float32)
    bv = np.asarray(bv, dtype=np.float32)
    Wo = np.asarray(Wo, dtype=np.float32)
    bo = np.asarray(bo, dtype=np.float32)

    nc = _get_nc()

    xT = {b: {"q": np.ascontiguousarray(query[b].T),
              "k": np.ascontiguousarray(key[b].T),
              "v": np.ascontiguousarray(value[b].T)} for b in range(B)}

    in_maps = []
    for c in range(N_CORES):
        b, g = divmod(c, TP)
        sl = slice(g * CH, (g + 1) * CH)
        in_maps.append({
            "xqT": xT[b]["q"], "xkT": xT[b]["k"], "xvT": xT[b]["v"],
            "wqT": np.ascontiguousarray(Wq[sl, :].T),
            "wkT": np.ascontiguousarray(Wk[sl, :].T),
            "wvT": np.ascontiguousarray(Wv[sl, :].T),
            "bqc": bq[sl], "bkc": bk[sl], "bvc": bv[sl],
            "woT": np.ascontiguousarray(Wo[sl, :].T),
            "bor": np.ascontiguousarray(
                np.broadcast_to(bo[sl].reshape(1, CH), (128, CH))),
        })

    kwargs = {}
    if _trace:
        kwargs.update(trace=True,
                      trace_cores=_trace_cores or list(range(N_CORES)))
    res = run_bass_kernel_spmd(nc, in_maps, core_ids=list(range(N_CORES)),
                               **kwargs)

    full = np.empty((B, L, E), dtype=np.float32)
    for c in range(N_CORES):
        b, g = divmod(c, TP)
        full[b, :, g * CH:(g + 1) * CH] = res.results[c]["out"]

    if _trace:
        kernel.last_exec_ns = res.exec_time_ns
        kernel.last_results = res
    return full


# revision 26
# speedup vs baseline: 1.8559x; 1.8559x over previous
"""Trainium2 Bass kernel for multi-head attention (nn_AttentionWithDropout).

Reference computation (fp32):
    q = query @ Wq.T + bq ; k = key @ Wk.T + bk ; v = value @ Wv.T + bv
    per head: P = softmax(q k^T / sqrt(E)) ; o = P v
    out = concat_heads(o) @ Wo.T + bo

Sharding (8 cores): data-parallel over batch (2 groups of 4 cores) x
tensor-parallel over heads (4 heads / 256 channels per core, Megatron
column-sharded Wq/Wk/Wv).  Each core computes attention output transposed
(aoT, [chans, tok]) for its heads, an AllGather within the 4-core batch
group collects the full aoT [E, L], and each core then computes the
complete output for a 256-wide slice of output channels with a full
contraction over E (fc_out sharded over output columns; bias sliced the
same way).  The host only concatenates the per-core output shards.

Softmax skips the max-subtraction: energies are ~N(0, 0.25^2) for this
problem (|energy| < ~1.5), so exp() is numerically safe, and the row sum
is produced by an extra all-ones column appended to V (so PV matmul gives
[o | r] in one accumulation); normalization multiplies by 1/r.
"""

import os
import sys

sys.path.insert(0, "/opt/trn_rl_repo")

import numpy as np

# ---- problem constants (hardcoded per the harness contract) ----
B, L, E = 2, 2048, 1024
H, D = 16, 64
N_CORES = 8
TP = 4                  # cores per batch group (head-parallel)
CH = E // TP            # 256 channels (4 heads) per core
LQ = L // TP            # 512 output tokens per core
SCALE = 1.0 / 32.0      # 1/sqrt(E)
KT = E // 128           # 8 contraction tiles for projections
NKT = L // 128          # 16 key-token tiles

# matmul operand dtype: "f32" (exact, 4 cyc/row) or "f32r" (fast fp32, 1
# cyc/row for moving dim >= 256, ~19-bit mantissa) -- matmul-operand tiles
# are allocated in this dtype so producers emit properly rounded values.
MM_DTYPE = os.environ.get("KERNEL_MM_DTYPE", "f32")
# custom-DVE ops (reciprocal_approx_*) are InstISA, which this walrus
# rejects ("ISA wrong length") -- keep the plain DVE reciprocal.
FAST_RECIP = os.environ.get("KERNEL_FAST_RECIP", "0") == "1"
AG_SPLIT = os.environ.get("KERNEL_AG_SPLIT", "1") == "1"


def _split_multi_waits(nc):
    """The nix walrus in this container only encodes one semaphore wait per
    instruction (setupSyncWait raises "Too many sync wait commands" above
    that).  Tile's wait assignment attaches several.  Hoist the extras into
    standalone InstEventSemaphore waits (the encoding `engine.wait_ge` uses)
    immediately before the owning instruction, preserving per-engine order
    and exact semantics."""
    from concourse import mybir

    n_split = 0
    for fn in nc.m.functions:
        for bb in fn.blocks:
            out = []
            for inst in bb.instructions:
                si = inst.sync_info
                if si is not None and si.on_wait and len(si.on_wait) > 1:
                    waits = list(si.on_wait)
                    for k, w in enumerate(waits[:-1]):
                        wi = mybir.InstEventSemaphore(
                            name=f"{inst.name}-hw{k}", ins=[], outs=[])
                        wi.engine = inst.engine
                        wi.debug = inst.debug
                        wi.sync_info = mybir.SyncInfo(on_wait=[w],
                                                      on_update=[])
                        out.append(wi)
                        n_split += 1
                    si.on_wait = [waits[-1]]
                out.append(inst)
            bb.instructions[:] = out
    return n_split


def _build_nc():
    import concourse.bass as bass
    import concourse.tile as tile
    from concourse import masks, mybir

    f32 = mybir.dt.float32
    mdt = mybir.dt.float32r if MM_DTYPE == "f32r" else f32
    AF = mybir.ActivationFunctionType

    def mm_in(ap):
        # DRAM tensors are declared fp32; loads into f32r tiles bitcast the
        # source so HWDGE sees matching dtypes (same 4-byte layout).
        return ap.bitcast(mdt) if mdt is not f32 else ap

    nc = bass.Bass("TRN2", target_bir_lowering=False, debug=False,
                   num_devices=N_CORES)

    # ---- per-core external IO ----
    xqT = nc.dram_tensor("xqT", [E, L], f32, kind="ExternalInput")
    xkT = nc.dram_tensor("xkT", [E, L], f32, kind="ExternalInput")
    xvT = nc.dram_tensor("xvT", [E, L], f32, kind="ExternalInput")
    wqT = nc.dram_tensor("wqT", [E, CH], f32, kind="ExternalInput")
    wkT = nc.dram_tensor("wkT", [E, CH], f32, kind="ExternalInput")
    wvT = nc.dram_tensor("wvT", [E, CH], f32, kind="ExternalInput")
    bqc = nc.dram_tensor("bqc", [CH], f32, kind="ExternalInput")
    bkc = nc.dram_tensor("bkc", [CH], f32, kind="ExternalInput")
    bvc = nc.dram_tensor("bvc", [CH], f32, kind="ExternalInput")
    woT = nc.dram_tensor("woT", [E, CH], f32, kind="ExternalInput")
    bor = nc.dram_tensor("bor", [128, CH], f32, kind="ExternalInput")
    onesc = nc.dram_tensor("onesc", [128, 4], f32, kind="ExternalInput")
    out = nc.dram_tensor("out", [L, CH], f32, kind="ExternalOutput")

    with tile.TileContext(nc) as tc:
        with (
            tc.tile_pool(name="consts", bufs=1) as consts,
            tc.tile_pool(name="persist", bufs=1) as persist,
            tc.tile_pool(name="dram", bufs=1, space="DRAM") as dpool,
        ):
            # split the all-gather per head-pair half so the first one
            # overlaps the second half's attention compute
            n_ag = 2 if AG_SPLIT else 1
            agr = CH // n_ag            # rows per all-gather
            ag_in = [dpool.tile([agr, L], f32, name=f"agin{i}")
                     for i in range(n_ag)]
            ag_out = [dpool.tile([TP, agr, L], f32, name=f"agout{i}")
                      for i in range(n_ag)]

            # ---- constants ----
            ones_sb = consts.tile([1, 128], f32)
            nc.vector.memset(ones_sb[:], 1.0)
            ident = consts.tile([128, 128], f32)
            masks.make_identity(nc, ident[:])
            bo_bcast = consts.tile([128, CH], f32)
            nc.sync.dma_start(bo_bcast[:], bor[:])

            bias_cols = {}
            for nm, src in (("q", bqc), ("k", bkc), ("v", bvc)):
                for ct in range(CH // 128):
                    t = consts.tile([128, 1], f32, name=f"b{nm}{ct}")
                    nc.sync.dma_start(
                        t[:], src[ct * 128:(ct + 1) * 128].unsqueeze(1))
                    bias_cols[(nm, ct)] = t

            # ---- persistent SBUF tensors ----
            qT = [persist.tile([128, L], mdt, name=f"qT{i}") for i in range(2)]
            kTt = [persist.tile([128, L], mdt, name=f"kT{i}") for i in range(2)]
            # v tiles: [tok 128, 4 heads x (64 v-cols + 1 ones-col)]
            v_sb = [persist.tile([128, 4, 65], mdt, name=f"v{t}")
                    for t in range(NKT)]
            for t in range(NKT):
                # ones column for the PV row-sum trick (memset can't write
                # f32r, so it comes from a host constant)
                nc.sync.dma_start(v_sb[t][:, :, 64:65],
                                  mm_in(onesc[:].unsqueeze(2)))
            woT_sb = [persist.tile([128, CH], mdt, name=f"woT{i}")
                      for i in range(KT)]
            for i in range(KT):
                nc.sync.dma_start(woT_sb[i][:],
                                  mm_in(woT[i * 128:(i + 1) * 128, :]))

            # ================= projections =================
            with (
                tc.tile_pool(name="xpool", bufs=9) as xpool,
                tc.tile_pool(name="wpool", bufs=9) as wpool,
                tc.tile_pool(name="evacp", bufs=3) as evacp,
                tc.tile_pool(name="ppj", bufs=2, space="PSUM") as ppj,
                tc.tile_pool(name="pvt", bufs=2, space="PSUM") as pvt,
            ):
                for nm, xT_d, wT_d in (("q", xqT, wqT), ("k", xkT, wkT),
                                       ("v", xvT, wvT)):
                    wch = []
                    for kt in range(KT):
                        w = wpool.tile([128, CH], mdt, name="wch")
                        nc.sync.dma_start(
                            w[:], mm_in(wT_d[kt * 128:(kt + 1) * 128, :]))
                        wch.append(w)
                    for tc5 in range(L // 512):
                        xch = []
                        for kt in range(KT):
                            x = xpool.tile([128, 512], mdt, name="xch")
                            nc.sync.dma_start(
                                x[:], mm_in(xT_d[kt * 128:(kt + 1) * 128,
                                                 tc5 * 512:(tc5 + 1) * 512]))
                            xch.append(x)
                        for ct in range(CH // 128):
                            ps = ppj.tile([128, 512], f32, name="pj")
                            for kt in range(KT):
                                nc.tensor.matmul(
                                    ps[:],
                                    wch[kt][:, ct * 128:(ct + 1) * 128],
                                    xch[kt][:],
                                    start=(kt == 0), stop=(kt == KT - 1))
                            if nm in ("q", "k"):
                                dst = (qT if nm == "q" else kTt)[ct]
                                nc.scalar.activation(
                                    dst[:, tc5 * 512:(tc5 + 1) * 512], ps[:],
                                    AF.Identity, bias=bias_cols[(nm, ct)][:])
                            else:
                                # v: add bias, then transpose to natural
                                # [tok, chan] layout with the ones column.
                                vt = evacp.tile([128, 512], f32, name="vtmp")
                                nc.scalar.activation(
                                    vt[:], ps[:], AF.Identity,
                                    bias=bias_cols[("v", ct)][:])
                                for i in range(4):
                                    tt = tc5 * 4 + i
                                    pt = pvt.tile([128, 128], f32, name="vt")
                                    nc.tensor.transpose(
                                        pt[:], vt[:, i * 128:(i + 1) * 128],
                                        ident[:])
                                    nc.vector.tensor_copy(
                                        v_sb[tt][:, 2 * ct:2 * ct + 2, 0:64],
                                        pt.rearrange("p (h d) -> p h d", h=2))

            # ================= attention =================
            with (
                tc.tile_pool(name="upool", bufs=3) as upool,
                tc.tile_pool(name="mpool", bufs=4) as mpool,
                tc.tile_pool(name="aotp", bufs=3) as aotp,
                tc.tile_pool(name="pst", bufs=2, space="PSUM") as pst,
                tc.tile_pool(name="pacc", bufs=2, space="PSUM") as pacc,
                tc.tile_pool(name="pbc", bufs=2, space="PSUM") as pbc,
            ):
                for hp in range(2):
                    for qc in range(TP):
                        accs = [pacc.tile([65, 512], f32, name="acc")
                                for _ in range(2)]
                        for kt in range(NKT):
                            st = pst.tile([128, 1024], f32, name="st")
                            for j in range(2):
                                nc.tensor.matmul(
                                    st[:, j * 512:(j + 1) * 512],
                                    kTt[hp][j * 64:(j + 1) * 64,
                                            kt * 128:(kt + 1) * 128],
                                    qT[hp][j * 64:(j + 1) * 64,
                                           qc * 512:(qc + 1) * 512],
                                    start=True, stop=True)
                            u = upool.tile([128, 1024], mdt, name="u")
                            nc.scalar.activation(u[:], st[:], AF.Exp,
                                                 scale=SCALE)
                            for j in range(2):
                                hl = 2 * hp + j
                                nc.tensor.matmul(
                                    accs[j][:],
                                    v_sb[kt][:, hl, :],
                                    u[:, j * 512:(j + 1) * 512],
                                    start=(kt == 0), stop=(kt == NKT - 1))
                        for j in range(2):
                            hl = 2 * hp + j
                            invr = mpool.tile([1, 512], f32, name="invr")
                            if FAST_RECIP:
                                nc.vector.reciprocal_approx_fast(
                                    invr[:], accs[j][64:65, :])
                            else:
                                nc.vector.reciprocal(invr[:],
                                                     accs[j][64:65, :])
                            bc = pbc.tile([64, 512], f32, name="bc")
                            nc.tensor.matmul(bc[:], ones_sb[:, 0:64],
                                             invr[:], start=True, stop=True)
                            bc_sb = mpool.tile([64, 512], f32, name="bcs")
                            nc.vector.tensor_copy(bc_sb[:], bc[:])
                            aoT_n = aotp.tile([64, 512], f32, name="aot")
                            nc.vector.tensor_mul(aoT_n[:], accs[j][0:64, :],
                                                 bc_sb[:])
                            agi = hp if AG_SPLIT else 0
                            row0 = hl * 64 - agi * agr
                            nc.sync.dma_start(
                                ag_in[agi][row0:row0 + 64,
                                           qc * 512:(qc + 1) * 512],
                                aoT_n[:])

                    # all-gather this half as soon as it is complete
                    if AG_SPLIT or hp == 1:
                        agi = hp if AG_SPLIT else 0
                        nc.gpsimd.collective_compute(
                            "AllGather", mybir.AluOpType.bypass,
                            replica_groups=[[0, 1, 2, 3], [4, 5, 6, 7]],
                            ins=[ag_in[agi].opt()], outs=[ag_out[agi].opt()])

            # ================= output projection =================
            # out[:, my 256 outchans] = aoT_full.T @ woT_slice + bo_slice
            # chan chunk kt=(src, half, row128) lives at ag_out[half][src].
            def ag_chunk(kt):
                if AG_SPLIT:
                    # half-major order: all of the (early) first all-gather's
                    # chunks before the second's, so accumulation can start
                    # while the second all-gather is still in flight.
                    h, src = divmod(kt, TP)
                    return ag_out[h][src, :, :], 2 * src + h
                src, row = divmod(kt, 2)
                return ag_out[0][src, row * 128:(row + 1) * 128, :], kt

            with (
                tc.tile_pool(name="agp", bufs=9) as agp,
                tc.tile_pool(name="opool", bufs=3) as opool,
                tc.tile_pool(name="pout", bufs=2, space="PSUM") as pout,
            ):
                for tcg in range(L // 512):
                    agch = []
                    for kt in range(KT):
                        src_ap, wi = ag_chunk(kt)
                        a = agp.tile([128, 512], mdt, name="agch")
                        nc.sync.dma_start(
                            a[:],
                            mm_in(src_ap[:, tcg * 512:(tcg + 1) * 512]))
                        agch.append((a, wi))
                    for ti in range(4):
                        ps = pout.tile([128, CH], f32, name="po")
                        for kt in range(KT):
                            a, wi = agch[kt]
                            nc.tensor.matmul(
                                ps[:],
                                a[:, ti * 128:(ti + 1) * 128],
                                woT_sb[wi][:],
                                start=(kt == 0), stop=(kt == KT - 1))
                        ob = opool.tile([128, CH], f32, name="ob")
                        nc.vector.tensor_add(ob[:], ps[:], bo_bcast[:])
                        row0 = tcg * 512 + ti * 128
                        nc.sync.dma_start(out[row0:row0 + 128, :], ob[:])

    _split_multi_waits(nc)
    return nc


_NC_CACHE = {}


def _get_nc():
    key = (MM_DTYPE, FAST_RECIP, AG_SPLIT)
    if key not in _NC_CACHE:
        _NC_CACHE[key] = _build_nc()
    return _NC_CACHE[key]


def kernel(query, key, value, Wq, bq, Wk, bk, Wv, bv, Wo, bo,
           _trace=False, _trace_cores=None):
    from concourse.bass_utils import run_bass_kernel_spmd

    query = np.asarray(query, dtype=np.float32)
    key = np.asarray(key, dtype=np.float32)
    value = np.asarray(value, dtype=np.float32)
    Wq = np.asarray(Wq, dtype=np.float32)
    bq = np.asarray(bq, dtype=np.float32)
    Wk = np.asarray(Wk, dtype=np.float32)
    bk = np.asarray(bk, dtype=np.float32)
    Wv = np.asarray(Wv, dtype=np.float32)
    bv = np.asarray(bv, dtype=np.float32)
    Wo = np.asarray(Wo, dtype=np.float32)
    bo = np.asarray(bo, dtype=np.float32)

    nc = _get_nc()

    xT = {b: {"q": np.ascontiguousarray(query[b].T),
              "k": np.ascontiguousarray(key[b].T),
              "v": np.ascontiguousarray(value[b].T)} for b in range(B)}

    in_maps = []
    for c in range(N_CORES):
        b, g = divmod(c, TP)
        sl = slice(g * CH, (g + 1) * CH)
        in_maps.append({
            "xqT": xT[b]["q"], "xkT": xT[b]["k"], "xvT": xT[b]["v"],
            "wqT": np.ascontiguousarray(Wq[sl, :].T),
            "wkT": np.ascontiguousarray(Wk[sl, :].T),
            "wvT": np.ascontiguousarray(Wv[sl, :].T),
            "bqc": bq[sl], "bkc": bk[sl], "bvc": bv[sl],
            "woT": np.ascontiguousarray(Wo[sl, :].T),
            "bor": np.ascontiguousarray(
                np.broadcast_to(bo[sl].reshape(1, CH), (128, CH))),
            "onesc": np.ones((128, 4), dtype=np.float32),
        })

    kwargs = {}
    if _trace:
        kwargs.update(trace=True,
                      trace_cores=_trace_cores or list(range(N_CORES)))
    res = run_bass_kernel_spmd(nc, in_maps, core_ids=list(range(N_CORES)),
                               **kwargs)

    full = np.empty((B, L, E), dtype=np.float32)
    for c in range(N_CORES):
        b, g = divmod(c, TP)
        full[b, :, g * CH:(g + 1) * CH] = res.results[c]["out"]

    if _trace:
        kernel.last_exec_ns = res.exec_time_ns
        kernel.last_results = res
    return full
